# revision 2
# baseline (speedup 1.0000x reference)
"""ViT-style dense transformer (12 blocks, dim 768, 199 tokens, B=32) on 8
Trainium2 NeuronCores.

Sharding: data-parallel over batch — 4 images per core, no collectives.

Device layout: activations are kept channels-major (x.T, shape [768, tokens])
so every GEMM uses the weight as the stationary operand directly. The
attention path runs in bf16 at exact 199-token width:
  - scores S[k, q] per (sample, head) with keys on partitions;
  - softmax denominator comes FREE from the attn@v matmul via an augmented
    v operand: stationary [keys, 128] holds v in 64 columns plus a ones
    column, so one matmul yields both O rows and the exp-sum row;
  - even heads place v at cols 0:64 (+ones at 64) -> O at partitions 0-63;
    odd heads place v at cols 64:128 (+ones at 0) -> O at partitions
    64-127, so the channels-major oT writes are partition-aligned and need
    no shift DMAs;
  - the sigmoid loc-mask multiplies v during the PE-transpose copy as a
    per-partition (= per-key) activation scale, which preserves the
    reference semantics (normalize first, then mask) because the ones
    column is left unscaled.
qkv/proj weights are bf16; fc1/fc2 stay float32r (full-rate fp32 for
free-dim >= 256) with fp32 PSUM accumulation. LayerNorm statistics are
ones-vector matmuls on the PE; affine params are folded into the following
weight matrices host-side; biases enter as K=1 bias-row matmuls.
"""
import os
import sys

sys.path.insert(0, "/opt/trn_rl_repo")

import ml_dtypes
import numpy as np

import concourse.bass as bass
import concourse.tile as tile
from concourse import bacc, mybir
from concourse.bass_utils import run_bass_kernel_spmd
from concourse.masks import make_identity

F32 = mybir.dt.float32
F32R = mybir.dt.float32r
BF16 = mybir.dt.bfloat16
AF = mybir.ActivationFunctionType
OP = mybir.AluOpType

N_CORES = 8
BL = 4            # samples per core
DEPTH = int(os.environ.get("KDEPTH", "12"))
MASK_START = int(os.environ.get("KMASKSTART", "9"))
HEADS, DIM, HD = 12, 768, 64
SCALE = HD ** -0.5
NTOK = 199        # tokens per sample
T = BL * NTOK     # 796 tokens per core
CT = DIM // 128   # 6 channel tiles
CH = (398, 398)   # token chunks for N<=512 matmuls
FCH = (512, 284)  # token chunks for the fused MLP (fc2 psum = 6 banks + fc1 2)
EPS = 1e-5


def chunk_off(c):
    return sum(CH[:c])


def _ln_stats(nc, ps, smpool, stpool, ones, invc, eps_t, x, xsq, c):
    """Per-token mean and rstd (both PSUM [128, cw] broadcasts) over the
    channel (partition x ct) axis of channels-major x, for token chunk c."""
    mm = nc.tensor.matmul
    act = nc.scalar.activation
    tt = nc.vector.tensor_tensor
    co, cw = chunk_off(c), CH[c]
    sraw = ps.tile([1, 398], F32, tag="ps")
    ssraw = ps.tile([1, 398], F32, tag="ps")
    for ct in range(CT):
        mm(sraw[:, :cw], ones[:, 0:1], x[:, ct, co:co + cw],
           start=(ct == 0), stop=(ct == CT - 1))
        mm(ssraw[:, :cw], ones[:, 0:1], xsq[:, ct, co:co + cw],
           start=(ct == 0), stop=(ct == CT - 1))
    srow = smpool.tile([1, 2, 398], F32R, tag="srow")
    act(srow[:, 0, :cw], sraw[:, :cw], AF.Copy)
    act(srow[:, 1, :cw], ssraw[:, :cw], AF.Copy)
    mu = ps.tile([128, 398], F32, tag="ps")
    msq = ps.tile([128, 398], F32, tag="ps")
    mm(mu[:, :cw], invc[:], srow[:, 0, :cw], start=True, stop=True)
    mm(msq[:, :cw], invc[:], srow[:, 1, :cw], start=True, stop=True)
    musq = stpool.tile([128, 398], F32, tag="lnsc")
    act(musq[:, :cw], mu[:, :cw], AF.Square)
    var = stpool.tile([128, 398], F32, tag="lnsc")
    tt(var[:, :cw], msq[:, :cw], musq[:, :cw], op=OP.subtract)
    sd = stpool.tile([128, 398], F32, tag="lnsc")
    act(sd[:, :cw], var[:, :cw], AF.Sqrt, bias=eps_t[:, 0:1])
    rstd = ps.tile([128, 398], F32, tag="ps")
    nc.vector.reciprocal(rstd[:, :cw], sd[:, :cw])
    return mu, rstd


def _ln_apply(nc, ps, smpool, stpool, ones, invc, eps_t, x, xsq, h):
    """h = (x - mu) * rstd, channels-major, chunk at a time."""
    tt = nc.vector.tensor_tensor
    for c in range(2):
        co, cw = chunk_off(c), CH[c]
        mu, rstd = _ln_stats(nc, ps, smpool, stpool, ones, invc, eps_t, x, xsq, c)
        for ct in range(CT):
            tt(h[:, ct, co:co + cw], x[:, ct, co:co + cw], mu[:, :cw],
               op=OP.subtract)
            tt(h[:, ct, co:co + cw], h[:, ct, co:co + cw], rstd[:, :cw],
               op=OP.mult)


def build_program():
    nc = bacc.Bacc("TRN2", target_bir_lowering=False, debug=False,
                   num_devices=N_CORES)

    # ---- DRAM parameters (per-core views, host-prepped) ----
    d_xT = nc.dram_tensor("xT", [DIM, BL * 196], F32R, kind="ExternalInput")
    d_pw = nc.dram_tensor("patch_wT", [CT, DIM, 128], F32R, kind="ExternalInput")
    d_qkvw = nc.dram_tensor("qkv_wp", [DEPTH, 18, DIM, 128], BF16,
                            kind="ExternalInput")
    d_bias = nc.dram_tensor("biasT", [DEPTH, 128, 54], F32, kind="ExternalInput")
    d_prw = nc.dram_tensor("proj_wp", [DEPTH, CT, DIM, 128], BF16,
                           kind="ExternalInput")
    d_f1w = nc.dram_tensor("fc1_wp", [DEPTH, 24, DIM, 128], F32R,
                           kind="ExternalInput")
    d_f2w = nc.dram_tensor("fc2_w", [DEPTH, 4 * DIM, DIM], F32R,
                           kind="ExternalInput")
    d_ones = nc.dram_tensor("ones_c", [128, 512], F32R, kind="ExternalInput")
    d_invc = nc.dram_tensor("invc_c", [1, 128], F32R, kind="ExternalInput")
    d_posc = nc.dram_tensor("posc", [DIM, 196], F32, kind="ExternalInput")
    d_extra = nc.dram_tensor("extra_cols", [DIM, 3], F32, kind="ExternalInput")
    d_fing = nc.dram_tensor("final_g", [128, CT], F32, kind="ExternalInput")
    d_finb = nc.dram_tensor("final_b", [128, CT], F32, kind="ExternalInput")
    d_out = nc.dram_tensor("out", [T, DIM], F32, kind="ExternalOutput")

    mm = nc.tensor.matmul
    act = nc.scalar.activation
    tt = nc.vector.tensor_tensor
    ts = nc.vector.tensor_scalar

    with tile.TileContext(nc) as tc:
        with (
            tc.tile_pool(name="const", bufs=1) as cpool,
            tc.tile_pool(name="x", bufs=1) as xpool,
            tc.tile_pool(name="big", bufs=2) as bigpool,    # xsq/h/h2 rotate
            tc.tile_pool(name="bigb", bufs=2) as bigbpool,  # vT/oT rotate (bf16)
            tc.tile_pool(name="qk", bufs=1) as qkpool,
            tc.tile_pool(name="v", bufs=1) as vpool,
            tc.tile_pool(name="aa", bufs=1) as aapool,
            tc.tile_pool(name="wmt", bufs=4) as wmtpool,    # [128,6,128] mt-slices
            tc.tile_pool(name="wkt", bufs=3) as wktpool,    # [128,768] k-slices
            tc.tile_pool(name="bias", bufs=1) as biaspool,
            tc.tile_pool(name="gelu", bufs=3) as gelupool,
            tc.tile_pool(name="stats", bufs=2) as stpool,
            tc.tile_pool(name="small", bufs=1) as smpool,
            tc.tile_pool(name="obuf", bufs=2) as obpool,
            tc.tile_pool(name="ps", bufs=8, space="PSUM") as ps,
        ):
            # ---------------- constants ----------------
            ones = cpool.tile([128, 512], F32R, tag="ones")
            nc.sync.dma_start(ones[:], d_ones[:])
            invc = cpool.tile([1, 128], F32R, tag="invc")
            nc.sync.dma_start(invc[:], d_invc[:])
            eps_t = cpool.tile([128, 1], F32, tag="eps")
            nc.vector.memset(eps_t[:], EPS)
            identr = cpool.tile([128, 128], F32R, tag="identr")
            make_identity(nc, identr[:])
            identb = cpool.tile([128, 128], BF16, tag="identb")
            make_identity(nc, identb[:])
            fing = cpool.tile([128, CT], F32, tag="fing")
            nc.sync.dma_start(fing[:], d_fing[:])
            finb = cpool.tile([128, CT], F32, tag="finb")
            nc.sync.dma_start(finb[:], d_finb[:])
            # rb broadcast selectors: [p, 0, 0:64]=1 (used from partition 64,
            # spreads an even head's recip row to out partitions 0-63);
            # [p, 1, 64:128]=1 (from partition 0, odd head -> 64-127).
            onesh = cpool.tile([128, 2, 128], BF16, tag="onesh")
            nc.vector.memset(onesh[:], 0.0)
            nc.vector.memset(onesh[:, 0, 0:64], 1.0)
            nc.vector.memset(onesh[:, 1, 64:128], 1.0)

            # residual stream, channels-major: x[p, ct, tok]
            x = xpool.tile([128, CT, T], F32R, tag="x")

            # augmented v, tokens-major: [keys, sample, headpair, 256] where
            # cols 0:64 = even-head v, 64 = even ones, 128 = odd ones,
            # 192:256 = odd-head v. vaug0 holds keys 0-127, vaug1 128-198.
            vaug0 = vpool.tile([128, BL, CT, 256], BF16, tag="vaug0")
            vaug1 = vpool.tile([128, BL, CT, 256], BF16, tag="vaug1")
            for vt in (vaug0, vaug1):
                nc.vector.memset(vt[:], 0.0)
                nc.vector.memset(vt[:, :, :, 64:65], 1.0)
                nc.vector.memset(vt[:, :, :, 128:129], 1.0)

            # ---------------- patch embed ----------------
            with tc.tile_pool(name="patch", bufs=1) as ppool:
                posc = ppool.tile([128, CT, 196], F32, tag="posc")
                nc.sync.dma_start(posc[:],
                                  d_posc.rearrange("(ct p) t -> p ct t", p=128))
                extra = ppool.tile([128, CT, 3], F32, tag="extra")
                nc.sync.dma_start(extra[:],
                                  d_extra.rearrange("(ct p) t -> p ct t", p=128))
                xt = bigpool.tile([128, CT, BL * 196], F32R, tag="big")
                nc.sync.dma_start(xt[:], d_xT.rearrange("(kt p) t -> p kt t", p=128))
                for mt in range(CT):
                    pw = wmtpool.tile([128, CT, 128], F32R, tag="wmtr")
                    nc.sync.dma_start(
                        pw[:], d_pw[mt].rearrange("(kt p) m -> p kt m", p=128))
                    for c in range(2):  # 392-token halves: samples (2c, 2c+1)
                        acc = ps.tile([128, 392], F32, tag="ps")
                        for kt in range(CT):
                            mm(acc[:], pw[:, kt, :],
                               xt[:, kt, c * 392:(c + 1) * 392],
                               start=(kt == 0), stop=(kt == CT - 1))
                        for si in range(2):
                            s = 2 * c + si
                            tt(x[:, mt, s * NTOK + 1: s * NTOK + 197],
                               acc[:, si * 196:(si + 1) * 196], posc[:, mt, :],
                               op=OP.add)
                for s in range(BL):
                    o = s * NTOK
                    nc.vector.tensor_copy(x[:, :, o:o + 1], extra[:, :, 0:1])
                    nc.vector.tensor_copy(x[:, :, o + 197:o + 199],
                                          extra[:, :, 1:3])

            # ---------------- transformer blocks ----------------
            for i in range(DEPTH):
                # ===== LN1 -> h (affine folded into qkv weights) =====
                xsq = bigpool.tile([128, CT, T], F32R, tag="big")
                act(xsq[:], x[:], AF.Square)
                h = bigpool.tile([128, CT, T], BF16, tag="bigh")
                _ln_apply(nc, ps, smpool, stpool, ones, invc, eps_t, x, xsq, h)

                # ===== qkv = h @ Wqkv (channels-major out) =====
                qk = qkpool.tile([128, 12, T], BF16, tag="qk")
                vT = bigbpool.tile([128, CT, T], BF16, tag="bigb")
                bt = biaspool.tile([128, 54], F32, tag="bt")
                nc.sync.dma_start(bt[:], d_bias[i])
                for mt in range(18):
                    wq = wmtpool.tile([128, CT, 128], BF16, tag="wmt")
                    nc.sync.dma_start(
                        wq[:], d_qkvw[i, mt].rearrange("(kt p) m -> p kt m", p=128))
                    for c in range(2):
                        co, cw = chunk_off(c), CH[c]
                        acc = ps.tile([128, 398], F32, tag="ps")
                        for kt in range(CT):
                            mm(acc[:, :cw], wq[:, kt, :], h[:, kt, co:co + cw],
                               start=(kt == 0), stop=(kt == CT - 1))
                        dst = (qk[:, mt, co:co + cw] if mt < 12
                               else vT[:, mt - 12, co:co + cw])
                        act(dst, acc[:, :cw], AF.Identity, bias=bt[:, mt:mt + 1])

                # ===== attention =====
                oT = bigbpool.tile([128, CT, T], BF16, tag="bigb")
                use_mask = i >= MASK_START
                for s in range(BL):
                    o = s * NTOK
                    aa = aapool.tile([128, 12, NTOK], BF16, tag="aa")
                    ab2 = aapool.tile([128, 12, NTOK], BF16, tag="ab2")
                    if use_mask:
                        macc0 = smpool.tile([128, 2], F32, tag="macc0")
                        macc1 = smpool.tile([128, 2], F32, tag="macc1")
                    # scores + exp (+ mask logit accumulation)
                    for h_ in range(12):
                        ro = (h_ % 2) * 64
                        jq, jk = h_ // 2, 6 + h_ // 2
                        for cc, mw in ((0, 128), (1, 71)):
                            S = ps.tile([128, NTOK], F32, tag="ps")
                            mm(S[:mw, :],
                               qk[ro:ro + 64, jk, o + cc * 128: o + cc * 128 + mw],
                               qk[ro:ro + 64, jq, o:o + NTOK],
                               start=True, stop=True)
                            at = (aa, ab2)[cc]
                            act(at[:mw, h_, :], S[:mw, :], AF.Exp, scale=SCALE)
                            if use_mask:
                                macc = (macc0, macc1)[cc]
                                if h_ == 0:
                                    nc.vector.tensor_copy(macc[:mw, :],
                                                          S[:mw, 197:199])
                                else:
                                    tt(macc[:mw, :], macc[:mw, :],
                                       S[:mw, 197:199], op=OP.add)
                    # loc-mask per key chunk
                    mks = []
                    if use_mask:
                        for cc, mw in ((0, 128), (1, 71)):
                            macc = (macc0, macc1)[cc]
                            sg = smpool.tile([128, 2], F32, tag="sg")
                            act(sg[:mw, :], macc[:mw, :], AF.Sigmoid,
                                scale=SCALE / 12)
                            mk = smpool.tile([128, 1], F32, tag=f"mk{cc}")
                            tt(mk[:mw, :], sg[:mw, 0:1], sg[:mw, 1:2], op=OP.max)
                            mks.append(mk)
                    # transpose v into augmented tokens-major layout,
                    # mask applied as per-key (= per-partition) scale
                    for cc, (vt, mw) in enumerate(((vaug0, 128), (vaug1, 71))):
                        for ct in range(CT):
                            ptr = ps.tile([128, 128], F32, tag="ps")
                            nc.tensor.transpose(
                                ptr[:mw, :],
                                vT[:, ct, o + cc * 128: o + cc * 128 + mw],
                                identb[:])
                            kw = ({"scale": mks[cc][:mw, 0:1]} if use_mask
                                  else {})
                            act(vt[:mw, s, ct, 0:64], ptr[:mw, 0:64],
                                AF.Copy, **kw)
                            act(vt[:mw, s, ct, 192:256], ptr[:mw, 64:128],
                                AF.Copy, **kw)
                    # attn@v with free denominator row; normalize via
                    # per-head broadcast matmul
                    rcp = smpool.tile([128, 12, NTOK], BF16, tag="rcp")
                    for h_ in range(12):
                        ro, hp = (h_ % 2) * 64, h_ // 2
                        voff = 0 if ro == 0 else 128
                        dro = 64 - ro          # denominator row: 64 even, 0 odd
                        Oh = ps.tile([128, NTOK], F32, tag="ps")
                        mm(Oh[:, :], vaug0[:, s, hp, voff:voff + 128],
                           aa[:, h_, :], start=True, stop=False)
                        mm(Oh[:, :], vaug1[:71, s, hp, voff:voff + 128],
                           ab2[:71, h_, :], start=False, stop=True)
                        with nc.allow_low_precision(reason="bf16 softmax denom"):
                            nc.vector.reciprocal(rcp[dro:dro + 1, h_, :],
                                                 Oh[dro:dro + 1, :])
                        rbh = ps.tile([128, NTOK], F32, tag="ps")
                        mm(rbh[:, :], onesh[dro:dro + 1, 1 - (h_ % 2), :],
                           rcp[dro:dro + 1, h_, :], start=True, stop=True)
                        act(oT[ro:ro + 64, hp, o:o + NTOK], Oh[ro:ro + 64, :],
                            AF.Copy)
                        tt(oT[ro:ro + 64, hp, o:o + NTOK],
                           oT[ro:ro + 64, hp, o:o + NTOK],
                           rbh[ro:ro + 64, :], op=OP.mult)

                # ===== proj + residual =====
                for mt in range(CT):
                    wp = wmtpool.tile([128, CT, 128], BF16, tag="wmt")
                    nc.sync.dma_start(
                        wp[:], d_prw[i, mt].rearrange("(kt p) m -> p kt m", p=128))
                    for c in range(2):
                        co, cw = chunk_off(c), CH[c]
                        acc = ps.tile([128, 398], F32, tag="ps")
                        for kt in range(CT):
                            mm(acc[:, :cw], wp[:, kt, :], oT[:, kt, co:co + cw],
                               start=(kt == 0), stop=(kt == CT - 1))
                        nc.vector.scalar_tensor_tensor(
                            x[:, mt, co:co + cw], acc[:, :cw],
                            bt[:, 18 + mt:19 + mt], x[:, mt, co:co + cw],
                            op0=OP.add, op1=OP.add)

                # ===== LN2 -> h2 =====
                xsq = bigpool.tile([128, CT, T], F32R, tag="big")
                act(xsq[:], x[:], AF.Square)
                h2 = bigpool.tile([128, CT, T], F32R, tag="big")
                _ln_apply(nc, ps, smpool, stpool, ones, invc, eps_t, x, xsq, h2)

                # ===== MLP: fc1 -> gelu -> fc2 (streamed, fc2 psum resident) ====
                for c in range(2):
                    co = sum(FCH[:c])
                    cw = FCH[c]
                    fc2acc = []
                    for mt in range(CT):
                        a2 = ps.tile([128, 512], F32, tag="ps")
                        fc2acc.append(a2)
                    for gmt in range(24):
                        wf = wmtpool.tile([128, CT, 128], F32R, tag="wmtr")
                        nc.sync.dma_start(
                            wf[:], d_f1w[i, gmt].rearrange("(kt p) m -> p kt m",
                                                           p=128))
                        a1 = ps.tile([128, 512], F32, tag="ps")
                        for kt in range(CT):
                            mm(a1[:, :cw], wf[:, kt, :], h2[:, kt, co:co + cw],
                               start=(kt == 0), stop=(kt == CT - 1))
                        g = gelupool.tile([128, 512], F32R, tag="gelu")
                        act(g[:, :cw], a1[:, :cw], AF.Gelu,
                            bias=bt[:, 30 + gmt:31 + gmt])
                        wf2 = wktpool.tile([128, DIM], F32R, tag="wkt")
                        nc.sync.dma_start(
                            wf2[:], d_f2w[i, gmt * 128:(gmt + 1) * 128, :])
                        for mt in range(CT):
                            mm(fc2acc[mt][:, :cw], wf2[:, mt * 128:(mt + 1) * 128],
                               g[:, :cw], start=(gmt == 0), stop=(gmt == 23))
                    for mt in range(CT):
                        nc.vector.scalar_tensor_tensor(
                            x[:, mt, co:co + cw], fc2acc[mt][:, :cw],
                            bt[:, 24 + mt:25 + mt], x[:, mt, co:co + cw],
                            op0=OP.add, op1=OP.add)

            # -------------- final LN + affine + transpose + out --------------
            xsq = bigpool.tile([128, CT, T], F32R, tag="big")
            act(xsq[:], x[:], AF.Square)
            xfin = bigpool.tile([128, CT, T], F32R, tag="big")
            _ln_apply(nc, ps, smpool, stpool, ones, invc, eps_t, x, xsq, xfin)
            for ct in range(CT):
                act(xfin[:, ct, :], xfin[:, ct, :], AF.Identity,
                    bias=finb[:, ct:ct + 1], scale=fing[:, ct:ct + 1])
            for t in range(7):
                tw = min(128, T - t * 128)
                ob = obpool.tile([128, DIM], F32, tag="ob")
                for ct in range(CT):
                    ptr = ps.tile([128, 128], F32, tag="ps")
                    nc.tensor.transpose(ptr[:tw, :],
                                        xfin[:, ct, t * 128:t * 128 + tw],
                                        identr[:])
                    act(ob[:tw, ct * 128:(ct + 1) * 128], ptr[:tw, :], AF.Copy)
                nc.sync.dma_start(d_out[t * 128:t * 128 + tw, :], ob[:tw, :])

    nc.compile()
    return nc


_CACHED = {}


def _prep_host(inputs):
    """Host-side sharding + layout prep (numpy only)."""
    f = np.float32
    bf = ml_dtypes.bfloat16
    x = inputs["x"]
    qkv_w = np.asarray(inputs["qkv_w"], f)
    ln1_g, ln1_b = np.asarray(inputs["ln1_g"], f), np.asarray(inputs["ln1_b"], f)
    ln2_g, ln2_b = np.asarray(inputs["ln2_g"], f), np.asarray(inputs["ln2_b"], f)
    fc1_w, fc1_b = np.asarray(inputs["fc1_w"], f), np.asarray(inputs["fc1_b"], f)
    fc2_w, fc2_b = np.asarray(inputs["fc2_w"], f), np.asarray(inputs["fc2_b"], f)
    proj_w, proj_b = np.asarray(inputs["proj_w"], f), np.asarray(inputs["proj_b"], f)

    shared = {}
    # patch conv as GEMM: lhsT [cpq, o] split into 6 mt-slices [768, 128]
    pwT = np.asarray(inputs["patch_w"], f).reshape(DIM, 768).T     # [cpq, o]
    shared["patch_wT"] = np.ascontiguousarray(
        pwT.reshape(DIM, CT, 128).transpose(1, 0, 2))
    # qkv weights with ln1 affine fold, mt-sliced
    qkvw = ln1_g[:, :, None] * qkv_w                               # [12,768,2304]
    shared["qkv_wp"] = np.ascontiguousarray(
        qkvw.reshape(12, DIM, 18, 128)[:DEPTH].transpose(0, 2, 1, 3)).astype(bf)
    attn_bias = np.einsum("dc,dco->do", ln1_b, qkv_w)              # [12, 2304]
    f1be = fc1_b + np.einsum("dc,dco->do", ln2_b, fc1_w)
    biasT = np.concatenate([
        attn_bias.reshape(12, 18, 128).transpose(0, 2, 1),
        proj_b.reshape(12, 6, 128).transpose(0, 2, 1),
        fc2_b.reshape(12, 6, 128).transpose(0, 2, 1),
        f1be.reshape(12, 24, 128).transpose(0, 2, 1),
    ], axis=2)                                                     # [12, 128, 54]
    shared["biasT"] = np.ascontiguousarray(biasT[:DEPTH]).astype(f)
    shared["proj_wp"] = np.ascontiguousarray(
        proj_w.reshape(12, DIM, CT, 128)[:DEPTH].transpose(0, 2, 1, 3)).astype(bf)
    f1w = ln2_g[:, :, None] * fc1_w
    shared["fc1_wp"] = np.ascontiguousarray(
        f1w.reshape(12, DIM, 24, 128)[:DEPTH].transpose(0, 2, 1, 3))
    shared["fc2_w"] = np.ascontiguousarray(fc2_w[:DEPTH])
    shared["ones_c"] = np.ones((128, 512), f)
    shared["invc_c"] = np.full((1, 128), 1.0 / DIM, f)
    shared["posc"] = np.ascontiguousarray(
        (np.asarray(inputs["pos_embed"], f)[0, 1:197]
         + np.asarray(inputs["patch_b"], f)[None, :]).T)
    extra = np.stack([
        np.asarray(inputs["cls_tok"], f)[0, 0] + np.asarray(inputs["pos_embed"], f)[0, 0],
        np.asarray(inputs["loc_tok"], f)[0, 0] + np.asarray(inputs["loc_embed"], f)[0, 0],
        np.asarray(inputs["loc_aug_tok"], f)[0, 0]
        + np.asarray(inputs["loc_aug_embed"], f)[0, 0],
    ], axis=1)
    shared["extra_cols"] = np.ascontiguousarray(extra).astype(f)
    shared["final_g"] = np.ascontiguousarray(
        np.asarray(inputs["norm_g"], f).reshape(CT, 128).T)
    shared["final_b"] = np.ascontiguousarray(
        np.asarray(inputs["norm_b"], f).reshape(CT, 128).T)

    in_maps = []
    for c in range(N_CORES):
        xs = np.asarray(x[BL * c:BL * (c + 1)], f)
        xT = np.ascontiguousarray(
            xs.reshape(BL, 3, 14, 16, 14, 16).transpose(1, 3, 5, 0, 2, 4)
            .reshape(DIM, BL * 196))
        m = dict(shared)
        m["xT"] = xT
        in_maps.append(m)
    return in_maps


def kernel(**inputs):
    if "nc" not in _CACHED:
        _CACHED["nc"] = build_program()
    nc = _CACHED["nc"]
    in_maps = _prep_host(inputs)
    res = run_bass_kernel_spmd(nc, in_maps, list(range(N_CORES)))
    out = np.concatenate([r["out"].reshape(BL, NTOK, DIM) for r in res.results],
                         axis=0)
    return out.astype(np.float32)
